# revision 42
# baseline (speedup 1.0000x reference)
"""Causal self-attention with RoPE, tensor-parallel over (batch, head-group)
across 8 NeuronCores.

Sharding: core c = 4*b + g handles batch b (of 2) and head group g (of 4),
i.e. heads 4g..4g+3.  Each core computes q/k projections in transposed
layout [head_dim, seq] (weights become matmul lhsT naturally), v in natural
layout [seq, head_dim], applies RoPE, runs causal attention without
max-subtraction (scores are O(3), exp is safe in fp32), and emits a partial
output projection.  The host sums the 4 per-head-group partials per batch.

All matmul operands are fp16 (full PE rate, f32 PSUM accumulation); the
non-matmul math (RoPE, exp, reciprocal) stays f32 or fp16 where safe.

v2 changes vs baseline:
- Startup: first-needed tensors (wq slab dt0, x tile 0) lead the sync DMA
  queue interleaved per-slab; bulk weights ride the scalar/gpsimd queues.
- Phase C: triangular scores/exp/AV on the block-diagonal (i >= j only),
  softmax denominator accumulated on the Vector engine in fp16 (frees
  ~150 PE matmuls), finished with one ones-matmul per (head, i-block) that
  also broadcasts across partitions (replaces the GpSimd broadcast).
- Phase C runs i-block-major and output-projection (phase D) matmul groups
  are spliced in as soon as each i-band's heads complete, hiding the exp
  (ACT) latency excess under projection matmuls.
- Output stored as fp16 (host accumulates partials in fp32).
"""

import sys
from contextlib import ExitStack

sys.path.insert(0, "/opt/trn_rl_repo")

import numpy as np

import concourse.bass as bass
import concourse.tile as tile
from concourse import bacc, bass_isa, mybir

B, S, D, H, HD = 2, 2048, 2048, 16, 128
NCORES = 8
HPC = H // 4  # heads per core = 4
DG = HPC * HD  # 512 cols per head group
P = 128
SB = 512  # s-block (matmul free dim)
NSB = S // SB  # 4
NDT = D // P  # 16 contraction tiles of the model dim
NST = S // P  # 16 seq tiles
F32 = mybir.dt.float32
F32R = mybir.dt.float32r
MMDT = mybir.dt.float16
MMNP = np.float16
SCALE = 1.0 / float(np.sqrt(HD))


def _build_program(with_qkv_bias: bool):
    nc = bacc.Bacc("TRN2", target_bir_lowering=False, debug=False,
                   num_devices=NCORES)
    # Weights arrive pre-tiled to the SBUF layout [partition, slab*cols] so
    # every DMA moves >=2KB-contiguous per-partition lines (1KB lines cap the
    # 16 DMA engines at ~250GB/s, below the PE's 296GB/s phase-A appetite).
    xT = nc.dram_tensor("xT", [D, S], MMDT, kind="ExternalInput").ap()
    wq = nc.dram_tensor("wq", [P, NDT * DG], MMDT, kind="ExternalInput").ap()
    wk = nc.dram_tensor("wk", [P, NDT * DG], MMDT, kind="ExternalInput").ap()
    wv = nc.dram_tensor("wv", [P, NDT * DG], MMDT, kind="ExternalInput").ap()
    wo = nc.dram_tensor("wo", [P, HPC * D], MMDT, kind="ExternalInput").ap()
    cosT = nc.dram_tensor("cosT", [P, S], MMDT, kind="ExternalInput").ap()
    sinST = nc.dram_tensor("sinST", [P, S], MMDT, kind="ExternalInput").ap()
    m128d = nc.dram_tensor("m128", [P, P], MMDT, kind="ExternalInput").ap()
    onesd = nc.dram_tensor("ones", [P, P], MMDT, kind="ExternalInput").ap()
    if with_qkv_bias:
        bqr = nc.dram_tensor("bqrope", [P, HPC, S], F32, kind="ExternalInput").ap()
        bkr = nc.dram_tensor("bkrope", [P, HPC, S], F32, kind="ExternalInput").ap()
        bv128 = nc.dram_tensor("bv128", [P, DG], F32, kind="ExternalInput").ap()
    out = nc.dram_tensor("out", [S, D], MMDT, kind="ExternalOutput").ap()

    with tile.TileContext(nc) as tc:
        with ExitStack() as top:
            # ---- persistent tiles ----
            qkT_pool = top.enter_context(tc.tile_pool(name="qkT", bufs=1))
            qT = qkT_pool.tile([P, HPC, S], MMDT, tag="qT")
            kT = qkT_pool.tile([P, HPC, S], MMDT, tag="kT")
            v_pool = top.enter_context(tc.tile_pool(name="vp", bufs=1))
            vN = v_pool.tile([P, NST, DG], MMDT, tag="vN")
            oT_pool = top.enter_context(tc.tile_pool(name="oTp", bufs=1))
            oT = oT_pool.tile([P, HPC, S], MMDT, tag="oT")
            wopool = top.enter_context(tc.tile_pool(name="wo", bufs=1))
            wo_t = wopool.tile([P, HPC, D], MMDT, tag="wo")
            mpool = top.enter_context(tc.tile_pool(name="masks", bufs=1))
            ones_t = mpool.tile([P, P], MMDT, tag="ones")
            m128_t = mpool.tile([P, P], MMDT, tag="m128")

            # ---- phase A: q/k (transposed) + v (natural) projections ----
            with ExitStack() as actx:
                wpool = actx.enter_context(tc.tile_pool(name="wqkv", bufs=1))
                wq_t = wpool.tile([P, NDT, DG], MMDT, tag="wq")
                wk_t = wpool.tile([P, NDT, DG], MMDT, tag="wk")
                wv_t = wpool.tile([P, NDT, DG], MMDT, tag="wv")
                cpool = actx.enter_context(tc.tile_pool(name="cs", bufs=1))
                cos_t = cpool.tile([P, S], MMDT, tag="cos")
                sin_t = cpool.tile([P, S], MMDT, tag="sin")
                xpool = actx.enter_context(tc.tile_pool(name="xs", bufs=20))
                tpool = actx.enter_context(tc.tile_pool(name="ropetmp", bufs=4))
                pspool = actx.enter_context(
                    tc.tile_pool(name="psA", bufs=8, space="PSUM"))

                # Need-ordered sync ring with 2KB+ per-partition lines:
                # wq 2-slab chunks interleaved with [P,1024] x tiles of the
                # first s-block pair (q wave consumes slab-by-slab), rope
                # tables mid-zip, then wk/wv halves.
                wqv = wq.rearrange("p (t n) -> p t n", n=DG)
                wkv = wk.rearrange("p (t n) -> p t n", n=DG)
                wvv = wv.rearrange("p (t n) -> p t n", n=DG)

                def load_xpair(sbp):
                    xts = []
                    for dt in range(NDT):
                        xt = xpool.tile([P, 2 * SB], MMDT, tag="xs",
                                        name=f"x_{sbp}_{dt}")
                        nc.sync.dma_start(
                            xt[:], xT[bass.ts(dt, P), bass.ts(sbp, 2 * SB)])
                        xts.append(xt)
                    return xts

                xts = []
                for dt in range(NDT):
                    if dt % 2 == 0:
                        nc.sync.dma_start(wq_t[:, dt:dt + 2, :],
                                          wqv[:, dt:dt + 2, :])
                    xt = xpool.tile([P, 2 * SB], MMDT, tag="xs",
                                    name=f"x_0_{dt}")
                    nc.sync.dma_start(xt[:],
                                      xT[bass.ts(dt, P), bass.ts(0, 2 * SB)])
                    xts.append(xt)
                    if dt == 7:
                        nc.sync.dma_start(cos_t[:, bass.ts(0, 2 * SB)],
                                          cosT[:, bass.ts(0, 2 * SB)])
                        nc.sync.dma_start(sin_t[:, bass.ts(0, 2 * SB)],
                                          sinST[:, bass.ts(0, 2 * SB)])
                nc.sync.dma_start(wk_t[:, 0:8, :], wkv[:, 0:8, :])
                nc.sync.dma_start(wk_t[:, 8:16, :], wkv[:, 8:16, :])
                nc.sync.dma_start(wv_t[:, 0:8, :], wvv[:, 0:8, :])
                nc.sync.dma_start(wv_t[:, 8:16, :], wvv[:, 8:16, :])
                nc.scalar.dma_start(ones_t[:], onesd[:])
                nc.scalar.dma_start(m128_t[:], m128d[:])
                if with_qkv_bias:
                    bpool = actx.enter_context(tc.tile_pool(name="bqk", bufs=1))
                    bqr_t = bpool.tile([P, HPC, S], F32, tag="bqr")
                    bkr_t = bpool.tile([P, HPC, S], F32, tag="bkr")
                    bv_t = bpool.tile([P, DG], F32, tag="bv")
                    nc.scalar.dma_start(bqr_t[:], bqr[:])
                    nc.scalar.dma_start(bkr_t[:], bkr[:])
                    nc.scalar.dma_start(bv_t[:], bv128[:])

                for sb in range(NSB):
                    ssl = bass.ts(sb, SB)
                    xoff = (sb % 2) * SB  # column window within the x pair
                    if sb % 2 == 1 and sb + 1 < NSB:
                        # prefetch next pair's x + rope tables during the
                        # second s-block of the current pair
                        nc.sync.dma_start(cos_t[:, bass.ts(sb // 2 + 1, 2 * SB)],
                                          cosT[:, bass.ts(sb // 2 + 1, 2 * SB)])
                        nc.sync.dma_start(sin_t[:, bass.ts(sb // 2 + 1, 2 * SB)],
                                          sinST[:, bass.ts(sb // 2 + 1, 2 * SB)])
                        next_xts = load_xpair(sb // 2 + 1)
                        if sb == 1:
                            # out-proj weights: on the sync ring behind the
                            # phase-A critical stream; needed only at ~200us
                            nc.sync.dma_start(
                                wo_t[:],
                                wo.rearrange("p (h n) -> p h n", n=D))

                    # wave q / wave k: transposed projection + RoPE
                    for wname, w_t, dst in (("q", wq_t, qT), ("k", wk_t, kT)):
                        ps = [pspool.tile([P, SB], F32, tag="psA",
                                          name=f"ps{wname}_{sb}_{h}")
                              for h in range(HPC)]
                        for dt in range(NDT):
                            for h in range(HPC):
                                nc.tensor.matmul(
                                    ps[h][:], w_t[:, dt, bass.ts(h, P)],
                                    xts[dt][:, xoff:xoff + SB],
                                    start=(dt == 0), stop=(dt == NDT - 1))
                        for h in range(HPC):
                            p = ps[h]
                            tmp = tpool.tile([P, SB], F32, tag="ropetmp")
                            nc.vector.tensor_mul(
                                tmp[0:64, :], p[64:128, :], sin_t[0:64, ssl])
                            nc.vector.tensor_mul(
                                tmp[64:128, :], p[0:64, :], sin_t[64:128, ssl])
                            dst_ap = dst[:, h, ssl]
                            nc.vector.tensor_mul(dst_ap, p[:], cos_t[:, ssl])
                            nc.vector.tensor_add(dst_ap, dst_ap, tmp[:])
                            if with_qkv_bias:
                                bt = bqr_t if wname == "q" else bkr_t
                                nc.vector.tensor_add(dst_ap, dst_ap,
                                                     bt[:, h, ssl])

                    # wave v: natural projection, lhsT is a slice of x
                    pv = [pspool.tile([P, DG], F32, tag="psA",
                                      name=f"psv_{sb}_{j}")
                          for j in range(4)]
                    for dt in range(NDT):
                        for j in range(4):
                            nc.tensor.matmul(
                                pv[j][:],
                                xts[dt][:, xoff + j * P:xoff + (j + 1) * P],
                                wv_t[:, dt, :],
                                start=(dt == 0), stop=(dt == NDT - 1))
                    for j in range(4):
                        st = 4 * sb + j
                        if with_qkv_bias:
                            nc.vector.tensor_add(vN[:, st, :], pv[j][:],
                                                 bv_t[:])
                        else:
                            nc.vector.tensor_copy(vN[:, st, :], pv[j][:])
                    if sb % 2 == 1 and sb + 1 < NSB:
                        xts = next_xts

            # ---- phase C + D interleaved ----
            # Attention runs i-block-major; each (head, i-block) does
            # triangular scores/exp/AV (columns i >= 128*jt only on the
            # diagonal band), denominator on DVE in fp16, and a final
            # ones-matmul that both reduces partitions and broadcasts.
            # Output-projection groups are spliced in as their i-band's oT
            # completes, keeping the PE busy while ACT works on exps.
            with ExitStack() as cctx:
                epool = cctx.enter_context(tc.tile_pool(name="et", bufs=6))
                dpool = cctx.enter_context(tc.tile_pool(name="dacc", bufs=2))
                rpool = cctx.enter_context(tc.tile_pool(name="recip", bufs=2))
                opool = cctx.enter_context(tc.tile_pool(name="outsb", bufs=6))
                psS = cctx.enter_context(
                    tc.tile_pool(name="psS", bufs=2, space="PSUM"))
                psO = cctx.enter_context(
                    tc.tile_pool(name="psO", bufs=2, space="PSUM"))
                psX = cctx.enter_context(
                    tc.tile_pool(name="psX", bufs=2, space="PSUM"))

                d_ready = []  # (st, eb) output-projection groups ready to go
                d_idx = [0]

                def emit_d(n, drain=False):
                    for k in range(n):
                        if d_idx[0] >= len(d_ready):
                            return
                        st, eb = d_ready[d_idx[0]]
                        d_idx[0] += 1
                        pe = psX.tile([P, SB], F32, tag="psX",
                                      name=f"pe_{st}_{eb}")
                        for hh in range(HPC):
                            nc.tensor.matmul(
                                pe[:], oT[:, hh, bass.ts(st, P)],
                                wo_t[:, hh, bass.ts(eb, SB)],
                                start=(hh == 0), stop=(hh == HPC - 1))
                        ob = opool.tile([P, SB], MMDT, tag="outsb")
                        # ACT evictions only in the final drain (no exps left
                        # to head-of-line-block in the strict-FIFO ACT queue)
                        if drain and k % 2 == 1:
                            nc.scalar.copy(ob[:], pe[:])
                        else:
                            nc.vector.tensor_copy(ob[:], pe[:])
                        nc.sync.dma_start(
                            out[bass.ts(st, P), bass.ts(eb, SB)], ob[:])

                def unlock_band(ib):
                    for st in range(4 * ib, 4 * ib + 4):
                        for eb in range(NSB):
                            d_ready.append((st, eb))

                # pending end-chain of the previous block: emitted after the
                # next block's first scores so the ones-matmul/reciprocal/
                # normalize never serialize against an idle PE
                pending = [None]

                def run_pending():
                    if pending[0] is not None:
                        fin, pib, last_head = pending[0]
                        pending[0] = None
                        fin()
                        if last_head:
                            unlock_band(pib)

                # ib=1 first: its longer blocks carry the PE across the A->C
                # boundary (no HAM re-throttle) and its bands unlock D-splice
                # filler for the short ib=0 blocks.  The pair pipeline is
                # FLAT across blocks: the next block's first scores issue
                # before the current block's last AV, so the last pair's exp
                # latency is always covered by real PE work.
                IBS = (1, 0, 2, 3)
                blocks = [(ib, h) for ib in IBS for h in range(HPC)]
                flat = []
                for bi, (ib, h) in enumerate(blocks):
                    flat += [(bi, pt) for pt in range(2 * (ib + 1))]
                bstate = {}

                def scores_of(bi, pt):
                    ib, h = blocks[bi]
                    if pt == 0:
                        bstate[bi] = {
                            "po": psO.tile([P, SB], F32, tag="psO",
                                           name=f"po_{h}_{ib}"),
                            "den": dpool.tile([P, SB], MMDT, tag="dacc",
                                              name=f"den_{h}_{ib}"),
                            "ets": {},
                            # block-end filler quota 2; odd-pair filler
                            # rationed so supply lasts into ib=3
                            "quota": {1: 0, 0: 0, 2: 2, 3: 4}[ib],
                        }
                    pss = psS.tile([P, 2, SB], F32, tag="psS",
                                   name=f"pss_{h}_{ib}_{pt}")
                    et = epool.tile([P, 2, SB], MMDT, tag="et",
                                    name=f"et_{h}_{ib}_{pt}")
                    i0s = []
                    for t in range(2):
                        jt = 2 * pt + t
                        i0 = max(0, P * (jt - 4 * ib))
                        i0s.append(i0)
                        nc.tensor.matmul(
                            pss[:, t, i0:SB],
                            kT[:, h, bass.ts(jt, P)],
                            qT[:, h, ib * SB + i0:(ib + 1) * SB],
                            start=True, stop=True)
                    i0a, i0b = i0s
                    if i0b == 0:  # fully sub-diagonal pair
                        nc.scalar.activation(
                            et[:], pss[:],
                            mybir.ActivationFunctionType.Exp, scale=SCALE)
                    else:  # diagonal pair: exp valid regions only
                        nc.scalar.activation(
                            et[:, :, i0b:SB], pss[:, :, i0b:SB],
                            mybir.ActivationFunctionType.Exp, scale=SCALE)
                        nc.scalar.activation(
                            et[:, 0, i0a:i0b], pss[:, 0, i0a:i0b],
                            mybir.ActivationFunctionType.Exp, scale=SCALE)
                        # triangular mask on the two 128-col wedges
                        nc.vector.tensor_mul(
                            et[:, 0, i0a:i0a + P],
                            et[:, 0, i0a:i0a + P], m128_t[:])
                        nc.vector.tensor_mul(
                            et[:, 1, i0b:i0b + P],
                            et[:, 1, i0b:i0b + P], m128_t[:])
                    bstate[bi]["ets"][pt] = et

                def av_den(bi, pt):
                    ib, h = blocks[bi]
                    stb = bstate[bi]
                    npair = 2 * (ib + 1)
                    njt = 2 * npair
                    po, den = stb["po"], stb["den"]
                    et = stb["ets"].pop(pt)
                    if pt == npair - 1:
                        emit_d(2)
                    for t in range(2):
                        jt = 2 * pt + t
                        i0 = max(0, P * (jt - 4 * ib))
                        nc.tensor.matmul(
                            po[:, i0:SB], vN[:, jt, bass.ts(h, P)],
                            et[:, t, i0:SB],
                            start=(jt == 0), stop=(jt == njt - 1))
                        if jt == 0:
                            nc.vector.tensor_copy(den[:], et[:, 0, :])
                        else:
                            nc.vector.tensor_add(den[:, i0:SB],
                                                 den[:, i0:SB],
                                                 et[:, t, i0:SB])
                    if pt % 2 == 1 and stb["quota"] > 0:
                        stb["quota"] -= 1
                        emit_d(1)
                    if pt == npair - 1:
                        isl = bass.ts(ib, SB)

                        # denominator end-chain, deferred into the next
                        # block: partition-reduce + broadcast in one
                        # ones-matmul, then reciprocal and normalize
                        def fin(po=po, den=den, h=h, isl=isl, ib=ib):
                            pd = psX.tile([P, SB], F32, tag="psX",
                                          name=f"pd_{h}_{ib}")
                            nc.tensor.matmul(pd[:], ones_t[:], den[:],
                                             start=True, stop=True)
                            recs = rpool.tile([P, SB], F32, tag="recs")
                            nc.vector.reciprocal_approx_fast(recs[:], pd[:])
                            nc.vector.tensor_mul(oT[:, h, isl], po[:],
                                                 recs[:])
                        pending[0] = (fin, ib, h == HPC - 1)

                scores_of(*flat[0])
                for i, (bi, pt) in enumerate(flat):
                    if i + 1 < len(flat):
                        scores_of(*flat[i + 1])
                    if pt == 0:
                        run_pending()
                    av_den(bi, pt)
                run_pending()
                emit_d(len(d_ready) - d_idx[0], drain=True)

    nc.compile()
    return nc


def _rot_cols(w):
    """rotate_half applied to the last axis (head-dim columns) of w."""
    r = np.empty_like(w)
    r[..., : HD // 2] = -w[..., HD // 2:]
    r[..., HD // 2:] = w[..., : HD // 2]
    return r


def _host_inputs(x, cos, sin, qkv_w, qkv_b, with_qkv_bias):
    """Build the 8 per-core input maps."""
    # signed sin, transposed: sinS[d] = -sin[d] for d<64 else +sin[d]
    sinS = sin.copy()
    sinS[:, : HD // 2] *= -1.0
    cosT = np.ascontiguousarray(cos.T).astype(MMNP)
    sinST = np.ascontiguousarray(sinS.T).astype(MMNP)
    jj = np.arange(P)[:, None]
    cc = np.arange(P)[None, :]
    m128 = (jj <= cc).astype(MMNP)  # keep j <= i within the diagonal tile
    ones = np.ones((P, P), dtype=MMNP)

    xTb = [np.ascontiguousarray(x[b].T).astype(MMNP) for b in range(B)]
    qkv_w16 = qkv_w.astype(MMNP)

    def tile_w(w):  # [D, DG] -> [P, NDT*DG]: slab-major per partition
        return np.ascontiguousarray(
            w.reshape(NDT, P, DG).transpose(1, 0, 2).reshape(P, NDT * DG))

    in_maps = []
    for c in range(NCORES):
        b, g = divmod(c, 4)
        cols = slice(g * DG, (g + 1) * DG)
        im = {
            "xT": xTb[b],
            "wq": tile_w(qkv_w16[:, cols]),
            "wk": tile_w(qkv_w16[:, D:][:, cols]),
            "wv": tile_w(qkv_w16[:, 2 * D:][:, cols]),
            "wo": None,  # filled by caller (needs out_w)
            "cosT": cosT,
            "sinST": sinST,
            "m128": m128,
            "ones": ones,
        }
        if with_qkv_bias:
            bq = qkv_b[cols]
            bk = qkv_b[D:][cols]
            bv = qkv_b[2 * D:][cols]
            # roped bias, transposed per head: [HD, HPC, S]
            def rope_bias(bvec):
                r = np.empty((P, HPC, S), dtype=np.float32)
                for h in range(HPC):
                    bh = bvec[h * HD:(h + 1) * HD]  # [HD]
                    rb = _rot_cols(bh[None, :])[0]
                    # b*cos + rot(b)*sin, as [HD, S]
                    r[:, h, :] = (bh[None, :] * cos + rb[None, :] * sin).T
                return r
            im["bqrope"] = rope_bias(bq)
            im["bkrope"] = rope_bias(bk)
            im["bv128"] = np.tile(bv[None, :], (P, 1)).astype(np.float32)
        in_maps.append(im)
    return in_maps


_CACHED = {}


def _get_program(with_qkv_bias):
    if with_qkv_bias not in _CACHED:
        _CACHED[with_qkv_bias] = _build_program(with_qkv_bias)
    return _CACHED[with_qkv_bias]


def run_on_cores(in_maps, profile_dir=None):
    """Execute the prebuilt program on 8 cores; optionally capture NTFF."""
    from concourse import bass2jax
    with_qkv_bias = "bqrope" in in_maps[0]
    nc = _get_program(with_qkv_bias)
    if profile_dir is not None:
        from trn_agent_boot.trn_boot import _ntff_profile_via_ctypes
        hook = _ntff_profile_via_ctypes("/opt/axon/libaxon_pjrt.so")
        with hook(profile_dir, [0]):
            results = bass2jax.run_bass_via_pjrt(nc, in_maps, n_cores=NCORES)
    else:
        results = bass2jax.run_bass_via_pjrt(nc, in_maps, n_cores=NCORES)
    return results


def kernel(x, cos, sin, qkv_w, qkv_b, out_w, out_b, _profile_dir=None):
    x = np.asarray(x, dtype=np.float32)
    cos = np.asarray(cos, dtype=np.float32)
    sin = np.asarray(sin, dtype=np.float32)
    qkv_w = np.asarray(qkv_w, dtype=np.float32)
    qkv_b = np.asarray(qkv_b, dtype=np.float32)
    out_w = np.asarray(out_w, dtype=np.float32)
    out_b = np.asarray(out_b, dtype=np.float32)

    with_qkv_bias = bool(np.any(qkv_b != 0))
    in_maps = _host_inputs(x, cos, sin, qkv_w, qkv_b, with_qkv_bias)
    for c in range(NCORES):
        g = c % 4
        woc = out_w[g * DG:(g + 1) * DG, :].astype(MMNP)  # [DG, D]
        # pre-tile to [P, HPC*D]: partition p holds head-rows p of each
        # 128-row group, concatenated along hh
        in_maps[c]["wo"] = np.ascontiguousarray(
            woc.reshape(HPC, P, D).transpose(1, 0, 2).reshape(P, HPC * D))

    results = run_on_cores(in_maps, profile_dir=_profile_dir)

    final = np.zeros((B, S, D), dtype=np.float32)
    for c in range(NCORES):
        b = c // 4
        final[b] += results[c]["out"].astype(np.float32)
    final += out_b[None, None, :]
    return final


# revision 43
# speedup vs baseline: 1.0139x; 1.0139x over previous
"""Causal self-attention with RoPE, tensor-parallel over (batch, head-group)
across 8 NeuronCores.

Sharding: core c = 4*b + g handles batch b (of 2) and head group g (of 4),
i.e. heads 4g..4g+3.  Each core computes q/k projections in transposed
layout [head_dim, seq] (weights become matmul lhsT naturally), v in natural
layout [seq, head_dim], applies RoPE, runs causal attention without
max-subtraction (scores are O(3), exp is safe in fp32), and emits a partial
output projection.  The host sums the 4 per-head-group partials per batch.

All matmul operands are fp16 (full PE rate, f32 PSUM accumulation); the
non-matmul math (RoPE, exp, reciprocal) stays f32 or fp16 where safe.

v2 changes vs baseline:
- Startup: first-needed tensors (wq slab dt0, x tile 0) lead the sync DMA
  queue interleaved per-slab; bulk weights ride the scalar/gpsimd queues.
- Phase C: triangular scores/exp/AV on the block-diagonal (i >= j only),
  softmax denominator accumulated on the Vector engine in fp16 (frees
  ~150 PE matmuls), finished with one ones-matmul per (head, i-block) that
  also broadcasts across partitions (replaces the GpSimd broadcast).
- Phase C runs i-block-major and output-projection (phase D) matmul groups
  are spliced in as soon as each i-band's heads complete, hiding the exp
  (ACT) latency excess under projection matmuls.
- Output stored as fp16 (host accumulates partials in fp32).
"""

import sys
from contextlib import ExitStack

sys.path.insert(0, "/opt/trn_rl_repo")

import numpy as np

import concourse.bass as bass
import concourse.tile as tile
from concourse import bacc, bass_isa, mybir

B, S, D, H, HD = 2, 2048, 2048, 16, 128
NCORES = 8
HPC = H // 4  # heads per core = 4
DG = HPC * HD  # 512 cols per head group
P = 128
SB = 512  # s-block (matmul free dim)
NSB = S // SB  # 4
NDT = D // P  # 16 contraction tiles of the model dim
NST = S // P  # 16 seq tiles
F32 = mybir.dt.float32
F32R = mybir.dt.float32r
MMDT = mybir.dt.float16
MMNP = np.float16
SCALE = 1.0 / float(np.sqrt(HD))


def _build_program(with_qkv_bias: bool):
    nc = bacc.Bacc("TRN2", target_bir_lowering=False, debug=False,
                   num_devices=NCORES)
    # Weights arrive pre-tiled to the SBUF layout [partition, slab*cols] so
    # every DMA moves >=2KB-contiguous per-partition lines (1KB lines cap the
    # 16 DMA engines at ~250GB/s, below the PE's 296GB/s phase-A appetite).
    xT = nc.dram_tensor("xT", [D, S], MMDT, kind="ExternalInput").ap()
    wq = nc.dram_tensor("wq", [P, NDT * DG], MMDT, kind="ExternalInput").ap()
    wk = nc.dram_tensor("wk", [P, NDT * DG], MMDT, kind="ExternalInput").ap()
    wv = nc.dram_tensor("wv", [P, NDT * DG], MMDT, kind="ExternalInput").ap()
    wo = nc.dram_tensor("wo", [P, HPC * D], MMDT, kind="ExternalInput").ap()
    cosT = nc.dram_tensor("cosT", [P, S], MMDT, kind="ExternalInput").ap()
    sinST = nc.dram_tensor("sinST", [P, S], MMDT, kind="ExternalInput").ap()
    m128d = nc.dram_tensor("m128", [P, P], MMDT, kind="ExternalInput").ap()
    onesd = nc.dram_tensor("ones", [P, P], MMDT, kind="ExternalInput").ap()
    if with_qkv_bias:
        bqr = nc.dram_tensor("bqrope", [P, HPC, S], F32, kind="ExternalInput").ap()
        bkr = nc.dram_tensor("bkrope", [P, HPC, S], F32, kind="ExternalInput").ap()
        bv128 = nc.dram_tensor("bv128", [P, DG], F32, kind="ExternalInput").ap()
    out = nc.dram_tensor("out", [S, D], MMDT, kind="ExternalOutput").ap()

    with tile.TileContext(nc) as tc:
        with ExitStack() as top:
            # ---- persistent tiles ----
            qkT_pool = top.enter_context(tc.tile_pool(name="qkT", bufs=1))
            qT = qkT_pool.tile([P, HPC, S], MMDT, tag="qT")
            kT = qkT_pool.tile([P, HPC, S], MMDT, tag="kT")
            v_pool = top.enter_context(tc.tile_pool(name="vp", bufs=1))
            vN = v_pool.tile([P, NST, DG], MMDT, tag="vN")
            oT_pool = top.enter_context(tc.tile_pool(name="oTp", bufs=1))
            oT = oT_pool.tile([P, HPC, S], MMDT, tag="oT")
            wopool = top.enter_context(tc.tile_pool(name="wo", bufs=1))
            wo_t = wopool.tile([P, HPC, D], MMDT, tag="wo")
            mpool = top.enter_context(tc.tile_pool(name="masks", bufs=1))
            ones_t = mpool.tile([P, P], MMDT, tag="ones")
            m128_t = mpool.tile([P, P], MMDT, tag="m128")

            # ---- phase A: q/k (transposed) + v (natural) projections ----
            with ExitStack() as actx:
                wpool = actx.enter_context(tc.tile_pool(name="wqkv", bufs=1))
                wq_t = wpool.tile([P, NDT, DG], MMDT, tag="wq")
                wk_t = wpool.tile([P, NDT, DG], MMDT, tag="wk")
                wv_t = wpool.tile([P, NDT, DG], MMDT, tag="wv")
                cpool = actx.enter_context(tc.tile_pool(name="cs", bufs=1))
                cos_t = cpool.tile([P, S], MMDT, tag="cos")
                sin_t = cpool.tile([P, S], MMDT, tag="sin")
                xpool = actx.enter_context(tc.tile_pool(name="xs", bufs=20))
                tpool = actx.enter_context(tc.tile_pool(name="ropetmp", bufs=4))
                pspool = actx.enter_context(
                    tc.tile_pool(name="psA", bufs=8, space="PSUM"))

                # Need-ordered sync ring with 2KB+ per-partition lines:
                # wq 2-slab chunks interleaved with [P,1024] x tiles of the
                # first s-block pair (q wave consumes slab-by-slab), rope
                # tables mid-zip, then wk/wv halves.
                wqv = wq.rearrange("p (t n) -> p t n", n=DG)
                wkv = wk.rearrange("p (t n) -> p t n", n=DG)
                wvv = wv.rearrange("p (t n) -> p t n", n=DG)

                def load_xpair(sbp):
                    xts = []
                    for dt in range(NDT):
                        xt = xpool.tile([P, 2 * SB], MMDT, tag="xs",
                                        name=f"x_{sbp}_{dt}")
                        nc.sync.dma_start(
                            xt[:], xT[bass.ts(dt, P), bass.ts(sbp, 2 * SB)])
                        xts.append(xt)
                    return xts

                xts = []
                for dt in range(NDT):
                    if dt % 2 == 0:
                        nc.sync.dma_start(wq_t[:, dt:dt + 2, :],
                                          wqv[:, dt:dt + 2, :])
                    xt = xpool.tile([P, 2 * SB], MMDT, tag="xs",
                                    name=f"x_0_{dt}")
                    nc.sync.dma_start(xt[:],
                                      xT[bass.ts(dt, P), bass.ts(0, 2 * SB)])
                    xts.append(xt)
                    if dt == 7:
                        nc.sync.dma_start(cos_t[:, bass.ts(0, 2 * SB)],
                                          cosT[:, bass.ts(0, 2 * SB)])
                        nc.sync.dma_start(sin_t[:, bass.ts(0, 2 * SB)],
                                          sinST[:, bass.ts(0, 2 * SB)])
                nc.sync.dma_start(wk_t[:, 0:8, :], wkv[:, 0:8, :])
                nc.sync.dma_start(wk_t[:, 8:16, :], wkv[:, 8:16, :])
                nc.sync.dma_start(wv_t[:, 0:8, :], wvv[:, 0:8, :])
                nc.sync.dma_start(wv_t[:, 8:16, :], wvv[:, 8:16, :])
                nc.scalar.dma_start(ones_t[:], onesd[:])
                nc.scalar.dma_start(m128_t[:], m128d[:])
                if with_qkv_bias:
                    bpool = actx.enter_context(tc.tile_pool(name="bqk", bufs=1))
                    bqr_t = bpool.tile([P, HPC, S], F32, tag="bqr")
                    bkr_t = bpool.tile([P, HPC, S], F32, tag="bkr")
                    bv_t = bpool.tile([P, DG], F32, tag="bv")
                    nc.scalar.dma_start(bqr_t[:], bqr[:])
                    nc.scalar.dma_start(bkr_t[:], bkr[:])
                    nc.scalar.dma_start(bv_t[:], bv128[:])

                for sb in range(NSB):
                    ssl = bass.ts(sb, SB)
                    xoff = (sb % 2) * SB  # column window within the x pair
                    if sb % 2 == 1 and sb + 1 < NSB:
                        # prefetch next pair's x + rope tables during the
                        # second s-block of the current pair
                        nc.sync.dma_start(cos_t[:, bass.ts(sb // 2 + 1, 2 * SB)],
                                          cosT[:, bass.ts(sb // 2 + 1, 2 * SB)])
                        nc.sync.dma_start(sin_t[:, bass.ts(sb // 2 + 1, 2 * SB)],
                                          sinST[:, bass.ts(sb // 2 + 1, 2 * SB)])
                        next_xts = load_xpair(sb // 2 + 1)
                        if sb == 1:
                            # out-proj weights: on the sync ring behind the
                            # phase-A critical stream; needed only at ~200us
                            nc.sync.dma_start(
                                wo_t[:],
                                wo.rearrange("p (h n) -> p h n", n=D))

                    # wave q / wave k: transposed projection + RoPE
                    for wname, w_t, dst in (("q", wq_t, qT), ("k", wk_t, kT)):
                        ps = [pspool.tile([P, SB], F32, tag="psA",
                                          name=f"ps{wname}_{sb}_{h}")
                              for h in range(HPC)]
                        for dt in range(NDT):
                            for h in range(HPC):
                                nc.tensor.matmul(
                                    ps[h][:], w_t[:, dt, bass.ts(h, P)],
                                    xts[dt][:, xoff:xoff + SB],
                                    start=(dt == 0), stop=(dt == NDT - 1))
                        for h in range(HPC):
                            p = ps[h]
                            tmp = tpool.tile([P, SB], F32, tag="ropetmp")
                            nc.vector.tensor_mul(
                                tmp[0:64, :], p[64:128, :], sin_t[0:64, ssl])
                            nc.vector.tensor_mul(
                                tmp[64:128, :], p[0:64, :], sin_t[64:128, ssl])
                            dst_ap = dst[:, h, ssl]
                            nc.vector.tensor_mul(dst_ap, p[:], cos_t[:, ssl])
                            nc.vector.tensor_add(dst_ap, dst_ap, tmp[:])
                            if with_qkv_bias:
                                bt = bqr_t if wname == "q" else bkr_t
                                nc.vector.tensor_add(dst_ap, dst_ap,
                                                     bt[:, h, ssl])

                    # wave v: natural projection, lhsT is a slice of x
                    pv = [pspool.tile([P, DG], F32, tag="psA",
                                      name=f"psv_{sb}_{j}")
                          for j in range(4)]
                    for dt in range(NDT):
                        for j in range(4):
                            nc.tensor.matmul(
                                pv[j][:],
                                xts[dt][:, xoff + j * P:xoff + (j + 1) * P],
                                wv_t[:, dt, :],
                                start=(dt == 0), stop=(dt == NDT - 1))
                    for j in range(4):
                        st = 4 * sb + j
                        if with_qkv_bias:
                            nc.vector.tensor_add(vN[:, st, :], pv[j][:],
                                                 bv_t[:])
                        else:
                            nc.vector.tensor_copy(vN[:, st, :], pv[j][:])
                    if sb % 2 == 1 and sb + 1 < NSB:
                        xts = next_xts

            # ---- phase C + D interleaved ----
            # Attention runs i-block-major; each (head, i-block) does
            # triangular scores/exp/AV (columns i >= 128*jt only on the
            # diagonal band), denominator on DVE in fp16, and a final
            # ones-matmul that both reduces partitions and broadcasts.
            # Output-projection groups are spliced in as their i-band's oT
            # completes, keeping the PE busy while ACT works on exps.
            with ExitStack() as cctx:
                epool = cctx.enter_context(tc.tile_pool(name="et", bufs=6))
                dpool = cctx.enter_context(tc.tile_pool(name="dacc", bufs=2))
                rpool = cctx.enter_context(tc.tile_pool(name="recip", bufs=2))
                opool = cctx.enter_context(tc.tile_pool(name="outsb", bufs=6))
                psS = cctx.enter_context(
                    tc.tile_pool(name="psS", bufs=2, space="PSUM"))
                psO = cctx.enter_context(
                    tc.tile_pool(name="psO", bufs=2, space="PSUM"))
                psX = cctx.enter_context(
                    tc.tile_pool(name="psX", bufs=2, space="PSUM"))

                d_ready = []  # (st, eb) output-projection groups ready to go
                d_idx = [0]

                def emit_d(n, drain=False):
                    for k in range(n):
                        if d_idx[0] >= len(d_ready):
                            return
                        st, eb = d_ready[d_idx[0]]
                        d_idx[0] += 1
                        pe = psX.tile([P, SB], F32, tag="psX",
                                      name=f"pe_{st}_{eb}")
                        for hh in range(HPC):
                            nc.tensor.matmul(
                                pe[:], oT[:, hh, bass.ts(st, P)],
                                wo_t[:, hh, bass.ts(eb, SB)],
                                start=(hh == 0), stop=(hh == HPC - 1))
                        ob = opool.tile([P, SB], MMDT, tag="outsb")
                        # ACT evictions only in the final drain (no exps left
                        # to head-of-line-block in the strict-FIFO ACT queue)
                        if drain and k % 2 == 1:
                            nc.scalar.copy(ob[:], pe[:])
                        else:
                            nc.vector.tensor_copy(ob[:], pe[:])
                        nc.sync.dma_start(
                            out[bass.ts(st, P), bass.ts(eb, SB)], ob[:])

                def unlock_band(ib):
                    for st in range(4 * ib, 4 * ib + 4):
                        for eb in range(NSB):
                            d_ready.append((st, eb))

                # pending end-chain of the previous block: emitted after the
                # next block's first scores so the ones-matmul/reciprocal/
                # normalize never serialize against an idle PE
                pending = [None]

                def run_pending():
                    if pending[0] is not None:
                        fin, pib, last_head = pending[0]
                        pending[0] = None
                        fin()
                        if last_head:
                            unlock_band(pib)

                # ib=1 first: its longer blocks carry the PE across the A->C
                # boundary (no HAM re-throttle) and its bands unlock D-splice
                # filler for the short ib=0 blocks.  The pair pipeline is
                # FLAT across blocks: the next block's first scores issue
                # before the current block's last AV, so the last pair's exp
                # latency is always covered by real PE work.
                IBS = (1, 0, 2, 3)
                blocks = [(ib, h) for ib in IBS for h in range(HPC)]
                flat = []
                for bi, (ib, h) in enumerate(blocks):
                    flat += [(bi, pt) for pt in range(2 * (ib + 1))]
                bstate = {}

                def scores_of(bi, pt):
                    ib, h = blocks[bi]
                    if pt == 0:
                        bstate[bi] = {
                            "po": psO.tile([P, SB], F32, tag="psO",
                                           name=f"po_{h}_{ib}"),
                            "den": dpool.tile([P, SB], MMDT, tag="dacc",
                                              name=f"den_{h}_{ib}"),
                            "ets": {},
                            # block-end filler quota 2; odd-pair filler
                            # rationed so supply lasts into ib=3
                            "quota": {1: 0, 0: 0, 2: 2, 3: 4}[ib],
                        }
                    pss = psS.tile([P, 2, SB], F32, tag="psS",
                                   name=f"pss_{h}_{ib}_{pt}")
                    et = epool.tile([P, 2, SB], MMDT, tag="et",
                                    name=f"et_{h}_{ib}_{pt}")
                    i0s = []
                    for t in range(2):
                        jt = 2 * pt + t
                        i0 = max(0, P * (jt - 4 * ib))
                        i0s.append(i0)
                        nc.tensor.matmul(
                            pss[:, t, i0:SB],
                            kT[:, h, bass.ts(jt, P)],
                            qT[:, h, ib * SB + i0:(ib + 1) * SB],
                            start=True, stop=True)
                    i0a, i0b = i0s
                    if bi < 5 and pt == 0:
                        # first five blocks have no D-splice filler, so the
                        # block's first AV waits directly on this exp: split
                        # it per-tile to halve the critical latency
                        for t, i0 in ((0, i0a), (1, i0b)):
                            nc.scalar.activation(
                                et[:, t, i0:SB], pss[:, t, i0:SB],
                                mybir.ActivationFunctionType.Exp,
                                scale=SCALE)
                            if i0b > 0:
                                nc.vector.tensor_mul(
                                    et[:, t, i0:i0 + P],
                                    et[:, t, i0:i0 + P], m128_t[:])
                    elif i0b == 0:  # fully sub-diagonal pair
                        nc.scalar.activation(
                            et[:], pss[:],
                            mybir.ActivationFunctionType.Exp, scale=SCALE)
                    else:  # diagonal pair: exp valid regions only
                        nc.scalar.activation(
                            et[:, :, i0b:SB], pss[:, :, i0b:SB],
                            mybir.ActivationFunctionType.Exp, scale=SCALE)
                        nc.scalar.activation(
                            et[:, 0, i0a:i0b], pss[:, 0, i0a:i0b],
                            mybir.ActivationFunctionType.Exp, scale=SCALE)
                        # triangular mask on the two 128-col wedges
                        nc.vector.tensor_mul(
                            et[:, 0, i0a:i0a + P],
                            et[:, 0, i0a:i0a + P], m128_t[:])
                        nc.vector.tensor_mul(
                            et[:, 1, i0b:i0b + P],
                            et[:, 1, i0b:i0b + P], m128_t[:])
                    bstate[bi]["ets"][pt] = et

                def av_den(bi, pt):
                    ib, h = blocks[bi]
                    stb = bstate[bi]
                    npair = 2 * (ib + 1)
                    njt = 2 * npair
                    po, den = stb["po"], stb["den"]
                    et = stb["ets"].pop(pt)
                    if pt == npair - 1:
                        emit_d(2)
                    for t in range(2):
                        jt = 2 * pt + t
                        i0 = max(0, P * (jt - 4 * ib))
                        nc.tensor.matmul(
                            po[:, i0:SB], vN[:, jt, bass.ts(h, P)],
                            et[:, t, i0:SB],
                            start=(jt == 0), stop=(jt == njt - 1))
                        if jt == 0:
                            nc.vector.tensor_copy(den[:], et[:, 0, :])
                        else:
                            nc.vector.tensor_add(den[:, i0:SB],
                                                 den[:, i0:SB],
                                                 et[:, t, i0:SB])
                    if pt % 2 == 1 and stb["quota"] > 0:
                        stb["quota"] -= 1
                        emit_d(1)
                    if pt == npair - 1:
                        isl = bass.ts(ib, SB)

                        # denominator end-chain, deferred into the next
                        # block: partition-reduce + broadcast in one
                        # ones-matmul, then reciprocal and normalize
                        def fin(po=po, den=den, h=h, isl=isl, ib=ib):
                            pd = psX.tile([P, SB], F32, tag="psX",
                                          name=f"pd_{h}_{ib}")
                            nc.tensor.matmul(pd[:], ones_t[:], den[:],
                                             start=True, stop=True)
                            recs = rpool.tile([P, SB], F32, tag="recs")
                            nc.vector.reciprocal_approx_fast(recs[:], pd[:])
                            nc.vector.tensor_mul(oT[:, h, isl], po[:],
                                                 recs[:])
                        pending[0] = (fin, ib, h == HPC - 1)

                scores_of(*flat[0])
                for i, (bi, pt) in enumerate(flat):
                    if i + 1 < len(flat):
                        scores_of(*flat[i + 1])
                    if pt == 0:
                        run_pending()
                    av_den(bi, pt)
                run_pending()
                emit_d(len(d_ready) - d_idx[0], drain=True)

    nc.compile()
    return nc


def _rot_cols(w):
    """rotate_half applied to the last axis (head-dim columns) of w."""
    r = np.empty_like(w)
    r[..., : HD // 2] = -w[..., HD // 2:]
    r[..., HD // 2:] = w[..., : HD // 2]
    return r


def _host_inputs(x, cos, sin, qkv_w, qkv_b, with_qkv_bias):
    """Build the 8 per-core input maps."""
    # signed sin, transposed: sinS[d] = -sin[d] for d<64 else +sin[d]
    sinS = sin.copy()
    sinS[:, : HD // 2] *= -1.0
    cosT = np.ascontiguousarray(cos.T).astype(MMNP)
    sinST = np.ascontiguousarray(sinS.T).astype(MMNP)
    jj = np.arange(P)[:, None]
    cc = np.arange(P)[None, :]
    m128 = (jj <= cc).astype(MMNP)  # keep j <= i within the diagonal tile
    ones = np.ones((P, P), dtype=MMNP)

    xTb = [np.ascontiguousarray(x[b].T).astype(MMNP) for b in range(B)]
    qkv_w16 = qkv_w.astype(MMNP)

    def tile_w(w):  # [D, DG] -> [P, NDT*DG]: slab-major per partition
        return np.ascontiguousarray(
            w.reshape(NDT, P, DG).transpose(1, 0, 2).reshape(P, NDT * DG))

    in_maps = []
    for c in range(NCORES):
        b, g = divmod(c, 4)
        cols = slice(g * DG, (g + 1) * DG)
        im = {
            "xT": xTb[b],
            "wq": tile_w(qkv_w16[:, cols]),
            "wk": tile_w(qkv_w16[:, D:][:, cols]),
            "wv": tile_w(qkv_w16[:, 2 * D:][:, cols]),
            "wo": None,  # filled by caller (needs out_w)
            "cosT": cosT,
            "sinST": sinST,
            "m128": m128,
            "ones": ones,
        }
        if with_qkv_bias:
            bq = qkv_b[cols]
            bk = qkv_b[D:][cols]
            bv = qkv_b[2 * D:][cols]
            # roped bias, transposed per head: [HD, HPC, S]
            def rope_bias(bvec):
                r = np.empty((P, HPC, S), dtype=np.float32)
                for h in range(HPC):
                    bh = bvec[h * HD:(h + 1) * HD]  # [HD]
                    rb = _rot_cols(bh[None, :])[0]
                    # b*cos + rot(b)*sin, as [HD, S]
                    r[:, h, :] = (bh[None, :] * cos + rb[None, :] * sin).T
                return r
            im["bqrope"] = rope_bias(bq)
            im["bkrope"] = rope_bias(bk)
            im["bv128"] = np.tile(bv[None, :], (P, 1)).astype(np.float32)
        in_maps.append(im)
    return in_maps


_CACHED = {}


def _get_program(with_qkv_bias):
    if with_qkv_bias not in _CACHED:
        _CACHED[with_qkv_bias] = _build_program(with_qkv_bias)
    return _CACHED[with_qkv_bias]


def run_on_cores(in_maps, profile_dir=None):
    """Execute the prebuilt program on 8 cores; optionally capture NTFF."""
    from concourse import bass2jax
    with_qkv_bias = "bqrope" in in_maps[0]
    nc = _get_program(with_qkv_bias)
    if profile_dir is not None:
        from trn_agent_boot.trn_boot import _ntff_profile_via_ctypes
        hook = _ntff_profile_via_ctypes("/opt/axon/libaxon_pjrt.so")
        with hook(profile_dir, [0]):
            results = bass2jax.run_bass_via_pjrt(nc, in_maps, n_cores=NCORES)
    else:
        results = bass2jax.run_bass_via_pjrt(nc, in_maps, n_cores=NCORES)
    return results


def kernel(x, cos, sin, qkv_w, qkv_b, out_w, out_b, _profile_dir=None):
    x = np.asarray(x, dtype=np.float32)
    cos = np.asarray(cos, dtype=np.float32)
    sin = np.asarray(sin, dtype=np.float32)
    qkv_w = np.asarray(qkv_w, dtype=np.float32)
    qkv_b = np.asarray(qkv_b, dtype=np.float32)
    out_w = np.asarray(out_w, dtype=np.float32)
    out_b = np.asarray(out_b, dtype=np.float32)

    with_qkv_bias = bool(np.any(qkv_b != 0))
    in_maps = _host_inputs(x, cos, sin, qkv_w, qkv_b, with_qkv_bias)
    for c in range(NCORES):
        g = c % 4
        woc = out_w[g * DG:(g + 1) * DG, :].astype(MMNP)  # [DG, D]
        # pre-tile to [P, HPC*D]: partition p holds head-rows p of each
        # 128-row group, concatenated along hh
        in_maps[c]["wo"] = np.ascontiguousarray(
            woc.reshape(HPC, P, D).transpose(1, 0, 2).reshape(P, HPC * D))

    results = run_on_cores(in_maps, profile_dir=_profile_dir)

    final = np.zeros((B, S, D), dtype=np.float32)
    for c in range(NCORES):
        b = c // 4
        final[b] += results[c]["out"].astype(np.float32)
    final += out_b[None, None, :]
    return final
